# revision 16
# baseline (speedup 1.0000x reference)
"""CGCNN forward on 8 Trainium2 NeuronCores (Bass/Tile).

Strategy (edge parallelism, dst-sorted), v2:
  - node rows sharded: core c owns 2500 nodes, padded to 2560 = 20 tiles of
    128; degree-sorted snake deal balances per-tile edge counts across cores.
  - edges assigned to the dst-owning core, grouped by dst tile, packed into
    128-wide chunks; chunk counts equalized across cores (SPMD).
  - per layer: node projections afas/bfbs via fp8 DoubleRow matmuls
    (K=256 contracted in one pass); bfbs AllGathered as a replicated
    [20480, 512] fp8 table in HBM.
  - per dst tile: ONE batched indirect DMA gathers all the tile's src rows;
    per 128-edge chunk: cf/cs = e @ We via fp8 DoubleRow; afas[dst] via
    one-hot fp8 matmul accumulated in the same PSUM; psum evicted+src-added
    by DVE per chunk pair; sigmoid/exp/ln on ACT in 4-tile groups (amortizes
    act-table reloads); msg = f*s on DVE; segment-sum via one-hot^T matmul.
  - BN stats accumulate across tiles in PSUM; layer-0 stats AllReduce
    overlaps the next layer's h transposes; BN is folded into layer-1
    projection weights (runtime row scaling) so BN apply leaves the critical
    path; the last layer's BN is folded past readout pooling and its stats
    ride the readout AllReduce (one collective saved).
  - readout: graph mean-pool partials -> AllReduce -> affine(BN) ->
    softplus -> @W_fc -> softplus (replicated).
"""
import contextlib
import sys

for _p in ("/opt/trn_rl_repo", "/root/.axon_site/_ro/trn_rl_repo"):
    if _p not in sys.path:
        sys.path.insert(0, _p)

import numpy as np
import ml_dtypes

BF16 = ml_dtypes.bfloat16
FP8 = ml_dtypes.float8_e4m3

# problem constants (hardcoded per contest rules)
N = 20000
E = 200000
H = 256
IN_NODE = 256
IN_EDGE = 128
NG = 64
L = 2
BN_EPS = 1e-5

NCORES = 8
P = 128
NSH = N // NCORES          # 2500 real nodes per core
NTILES = (NSH + P - 1) // P  # 20
NPAD = NTILES * P          # 2560
TROWS = NPAD * NCORES      # 20480 rows in the replicated bfbs table
H2 = 2 * H                 # 512
GROUP = 5                  # dst tiles per gate group


def _prep(x, edge_index, edge_attr, batch, W_emb, b_emb, W_edge, b_edge,
          Wf, bf, Ws, bs, gamma, beta, W_fc, b_fc):
    """Host-side sharding prep. Returns (static_cfg, shared_inputs, per_core_inputs)."""
    x = np.asarray(x, np.float32)
    ei = np.asarray(edge_index).astype(np.int64)
    ea = np.asarray(edge_attr, np.float32)
    batch = np.asarray(batch).astype(np.int64)

    src, dst = ei[0], ei[1]

    # Node relabeling: deal degree-sorted nodes snake-wise across all
    # (core, tile) bins so per-tile edge counts are balanced across cores.
    NBINS = NCORES * NTILES
    deg_n = np.bincount(dst, minlength=N)
    order_nodes = np.argsort(-deg_n, kind="stable")
    ii = np.arange(N)
    rr = ii // NBINS
    pp = ii % NBINS
    bb = np.where(rr % 2 == 0, pp, NBINS - 1 - pp)
    core_n = np.empty(N, np.int64)
    tile_n = np.empty(N, np.int64)
    pos_n = np.empty(N, np.int64)
    core_n[order_nodes] = bb // NTILES
    tile_n[order_nodes] = bb % NTILES
    pos_n[order_nodes] = rr
    assert pos_n.max() < P

    core_of = core_n[dst]
    tile_of = tile_n[dst]
    pos_in_tile = pos_n[dst]

    # edge counts per (core, tile)
    cnt = np.zeros((NCORES, NTILES), np.int64)
    np.add.at(cnt, (core_of, tile_of), 1)
    KT = np.ceil(cnt.max(axis=0) / P).astype(np.int64)  # chunks per tile idx
    KT = np.maximum(KT, 1)
    NCHUNK = int(KT.sum())
    EP = NCHUNK * P
    chunk_base = np.concatenate([[0], np.cumsum(KT)])[:-1]  # first chunk of tile t

    # global src -> row in the replicated (padded) table
    srow_of = core_n[src] * NPAD + tile_n[src] * P + pos_n[src]

    order = np.lexsort((tile_of, core_of))
    deg = np.bincount(dst, minlength=N).astype(np.float32)
    inv_deg_full = 1.0 / np.maximum(deg, 1.0)

    gsz = np.bincount(batch, minlength=NG).astype(np.float32)
    inv_gsz = (1.0 / np.maximum(gsz, 1.0)).astype(np.float32).reshape(NG, 1)

    per_core = []
    for c in range(NCORES):
        sel = order[core_of[order] == c]
        t_sel = tile_of[sel]
        attrT = np.zeros((IN_EDGE, EP), np.float32)
        og = np.zeros((P, NCHUNK, P), np.float32)   # [node_in_tile, chunk, edge]
        osc = np.zeros((P, NCHUNK, P), np.float32)  # [edge, chunk, node_in_tile]
        srow = np.zeros((P, NCHUNK), np.int32)      # [edge_in_chunk, chunk]

        for t in range(NTILES):
            e_t = sel[t_sel == t]
            n_t = e_t.size
            assert n_t <= KT[t] * P, (c, t, n_t, KT[t] * P)
            j = np.arange(n_t)
            q = chunk_base[t] + j // P   # chunk id
            e_in = j % P                 # position in chunk
            n_in = pos_in_tile[e_t]      # dst position within tile
            og[n_in, q, e_in] = 1.0
            osc[e_in, q, n_in] = 1.0
            srow[e_in, q] = srow_of[e_t]
            attrT[:, q * P + e_in] = ea[e_t].T

        # in_edge feature f -> (half i = f // 64, partition p = f % 64)
        attrT8 = np.ascontiguousarray(
            attrT.reshape(2, 64, EP).transpose(1, 0, 2)).astype(FP8)

        mine = np.nonzero(core_n == c)[0]
        tl, ps = tile_n[mine], pos_n[mine]
        xs = np.zeros((NPAD, IN_NODE), np.float32)
        xs[tl * P + ps] = x[mine]
        xT = np.ascontiguousarray(
            xs.T.reshape(2, P, NPAD).transpose(1, 0, 2)).astype(BF16)

        invdeg = np.zeros((P, NTILES), np.float32)
        invdeg[ps, tl] = inv_deg_full[mine]

        onesmask = np.zeros((P, NTILES), np.float32)
        onesmask[ps, tl] = 1.0

        oggraph = np.zeros((P, NTILES, NG), np.float32)
        oggraph[ps, tl, batch[mine]] = 1.0

        per_core.append(dict(
            xT=xT,
            attrT8=attrT8,
            og=np.ascontiguousarray(og).astype(FP8),
            osc=np.ascontiguousarray(osc).astype(BF16),
            srow=np.ascontiguousarray(srow),
            invdeg=invdeg,
            onesmask=onesmask.astype(BF16),
            oggraph=oggraph.astype(BF16),
            invgsz=inv_gsz,
        ))

    # shared (replicated) weights
    Wf = np.asarray(Wf, np.float32)
    Ws = np.asarray(Ws, np.float32)
    bf = np.asarray(bf, np.float32)
    bs = np.asarray(bs, np.float32)
    W_emb = np.asarray(W_emb, np.float32)
    W_edge = np.asarray(W_edge, np.float32)
    b_edge = np.asarray(b_edge, np.float32)
    b_emb = np.asarray(b_emb, np.float32)
    gamma = np.asarray(gamma, np.float32)
    beta = np.asarray(beta, np.float32)
    W_fc = np.asarray(W_fc, np.float32)
    b_fc = np.asarray(b_fc, np.float32)

    def pack_cat(Wl_f, Wl_s):
        # [256, 512]: col j<H -> -Wl_f[j, :]; j>=H -> Wl_s[j-H, :]
        # f-half negated: sigmoid computed with scale=-1.
        return np.concatenate([-Wl_f.T, Wl_s.T], axis=1)

    def split_k(r, dt):
        # [K, F] -> [K//128, ...] packed [128, K//128, F] with k = i*128+p
        K = r.shape[0]
        return np.ascontiguousarray(
            r.reshape(K // P, P, r.shape[1]).transpose(1, 0, 2)).astype(dt)

    P_i = [pack_cat(Wf[l][:, :H], Ws[l][:, :H]) for l in range(L)]
    P_j = [pack_cat(Wf[l][:, H:2 * H], Ws[l][:, H:2 * H]) for l in range(L)]

    shared = dict(
        W_embT=np.ascontiguousarray(
            W_emb.T.reshape(2, P, H).transpose(1, 0, 2)).astype(BF16),
        W_fcT=np.ascontiguousarray(
            W_fc.T.reshape(2, P, H).transpose(1, 0, 2)).astype(BF16),
        onescol=np.ones((1, P), np.float32).astype(BF16),
        gamma=gamma.reshape(L, 1, H),
        beta=beta.reshape(L, 1, H),
        b_embrow=b_emb.reshape(1, H),
        b_fcrow=b_fc.reshape(1, H),
        Wsum1=split_k((P_i[L - 1] + P_j[L - 1]), BF16),  # [128, 2, 512]
    )
    for l in range(L):
        shared[f"WiT{l}"] = split_k(P_i[l], FP8)
        shared[f"WjT{l}"] = split_k(P_j[l], FP8)
        # compose edge projection with the attr embedding, pack K=128 as 64x2
        We_cat = np.concatenate([
            -(Wf[l][:, 2 * H:].astype(np.float64) @ W_edge.astype(np.float64)).T,
            (Ws[l][:, 2 * H:].astype(np.float64) @ W_edge.astype(np.float64)).T,
        ], axis=1)  # [128, 512]
        shared[f"WeT{l}"] = np.ascontiguousarray(
            We_cat.reshape(2, 64, H2).transpose(1, 0, 2)).astype(FP8)
        brow = np.concatenate([
            -(bf[l] + Wf[l][:, 2 * H:] @ b_edge),
            bs[l] + Ws[l][:, 2 * H:] @ b_edge,
        ]).astype(np.float32).reshape(1, H2)
        shared[f"brow{l}"] = brow.astype(BF16)

    cfg = dict(KT=[int(k) for k in KT], NCHUNK=NCHUNK, EP=EP,
               chunk_base=[int(b) for b in chunk_base],
               has_emb_bias=bool(np.any(b_emb != 0)),
               has_brow=[bool(np.any(np.abs(shared[f"brow{l}"].astype(np.float32)) > 0))
                         for l in range(L)],
               has_fc_bias=bool(np.any(b_fc != 0)))
    return cfg, shared, per_core


def _patch_act_tables():
    """Make the act-table chooser pick natural_log_exp_and_others for both
    Exp and Ln (greedy-first-match otherwise ping-pongs two tables per
    chunk, costing ~1.3us per reload)."""
    import concourse.bacc as bacc_mod
    from concourse import mybir
    from concourse.hw_specs import get_activation_tables as _orig_gat
    if getattr(bacc_mod, "_act_tables_patched", False):
        return
    AF = mybir.ActivationFunctionType

    def _patched(arch):
        tabs = _orig_gat(arch)
        for name, fns in tabs.items():
            if name != "natural_log_exp_and_others":
                fns.discard(AF.Exp)
                fns.discard(AF.Ln)
        return tabs

    bacc_mod.get_activation_tables = _patched
    bacc_mod._act_tables_patched = True


def _build(cfg, reps=1, sim=False):
    """Build the Bass program (same for all cores)."""
    from concourse import bass, bacc, tile, mybir
    from concourse.masks import make_identity
    _patch_act_tables()

    KT = cfg["KT"]
    NCHUNK = cfg["NCHUNK"]
    EP = cfg["EP"]
    chunk_base = cfg["chunk_base"]
    KTMAX = max(KT)
    fp32 = mybir.dt.float32
    bf16 = mybir.dt.bfloat16
    fp8 = mybir.dt.float8e4
    i32 = mybir.dt.int32
    AF = mybir.ActivationFunctionType
    OP = mybir.AluOpType
    DR = mybir.MatmulPerfMode.DoubleRow

    nc = bacc.Bacc("TRN2", target_bir_lowering=False, debug=False,
                   num_devices=1 if sim else NCORES)

    def din(name, shape, dt):
        return nc.dram_tensor(name, list(shape), dt, kind="ExternalInput").ap()

    # per-core inputs
    xT = din("xT", [P, 2, NPAD], bf16)
    attrT8 = din("attrT8", [64, 2, EP], fp8)
    og = din("og", [P, NCHUNK, P], fp8)
    osc = din("osc", [P, NCHUNK, P], bf16)
    srow = din("srow", [P, NCHUNK], i32)
    invdeg = din("invdeg", [P, NTILES], fp32)
    onesmask = din("onesmask", [P, NTILES], bf16)
    oggraph = din("oggraph", [P, NTILES, NG], bf16)
    invgsz = din("invgsz", [NG, 1], fp32)
    # shared weights
    W_embT = din("W_embT", [P, 2, H], bf16)
    W_fcT = din("W_fcT", [P, 2, H], bf16)
    onescol = din("onescol", [1, P], bf16)
    gamma = din("gamma", [L, 1, H], fp32)
    beta = din("beta", [L, 1, H], fp32)
    b_embrow = din("b_embrow", [1, H], fp32)
    b_fcrow = din("b_fcrow", [1, H], fp32)
    Wsum1 = din("Wsum1", [P, 2, H2], bf16)
    WiT = [din(f"WiT{l}", [P, 2, H2], fp8) for l in range(L)]
    WjT = [din(f"WjT{l}", [P, 2, H2], fp8) for l in range(L)]
    WeT = [din(f"WeT{l}", [64, 2, H2], fp8) for l in range(L)]
    brow = [din(f"brow{l}", [1, H2], bf16) for l in range(L)]

    out = nc.dram_tensor("out", [NG, H], fp32, kind="ExternalOutput").ap()

    groups = [list(range(NCORES))]
    NGX = P  # readout AR payload: 64 graph rows + stats at partitions 64/96

    with tile.TileContext(nc) as tc:
        with tc.tile_pool(name="const", bufs=1) as const, \
             tc.tile_pool(name="state", bufs=1) as state, \
             tc.tile_pool(name="stream", bufs=3) as stream, \
             tc.tile_pool(name="work", bufs=3) as work, \
             tc.tile_pool(name="pre_ps", bufs=4, space="PSUM") as pre_pool, \
             tc.tile_pool(name="agg_ps", bufs=1, space="PSUM") as agg_pool, \
             tc.tile_pool(name="st_ps", bufs=1, space="PSUM") as st_pool, \
             tc.tile_pool(name="misc_ps", bufs=2, space="PSUM") as misc_pool, \
             tc.tile_pool(name="dram", bufs=1, space="DRAM") as dram:

            def misc_ps(shape, name):
                return misc_pool.tile(shape, fp32, tag="mps", name=name)

            # ---------- resident SBUF constants ----------
            def load_const(ap, dt=None, name=None):
                t = const.tile(list(ap.shape), dt or ap.dtype, name=name)
                nc.sync.dma_start(t[:], ap[:])
                return t

            W_embT_sb = load_const(W_embT, name="W_embT_sb")
            W_fcT_sb = load_const(W_fcT, name="W_fcT_sb")
            onescol_sb = load_const(onescol, name="onescol_sb")
            invdeg_sb = load_const(invdeg, name="invdeg_sb")
            onesmask_sb = load_const(onesmask, name="onesmask_sb")
            oggraph_sb = load_const(oggraph, name="oggraph_sb")
            invgsz_sb = load_const(invgsz, name="invgsz_sb")
            Wsum1_sb = load_const(Wsum1, name="Wsum1_sb")
            WiT_sb = [load_const(WiT[l], name=f"WiT_sb{l}") for l in range(L)]
            WjT_sb = [load_const(WjT[l], name=f"WjT_sb{l}") for l in range(L)]
            WeT_sb = [load_const(WeT[l], name=f"WeT_sb{l}") for l in range(L)]
            gamma_sb = []
            beta_sb = []
            for l in range(L):
                gt = const.tile([1, H], fp32, name=f"gamma_sb{l}")
                nc.sync.dma_start(gt[:], gamma[l, :, :])
                gamma_sb.append(gt)
                bt_ = const.tile([1, H], fp32, name=f"beta_sb{l}")
                nc.sync.dma_start(bt_[:], beta[l, :, :])
                beta_sb.append(bt_)
            b_embrow_sb = load_const(b_embrow, name="b_embrow_sb") \
                if cfg["has_emb_bias"] else None
            b_fcrow_sb = load_const(b_fcrow, name="b_fcrow_sb") \
                if cfg["has_fc_bias"] else None
            brow_sb = [load_const(brow[l], name=f"brow_sb{l}")
                       if cfg["has_brow"][l] else None for l in range(L)]

            ident = const.tile([P, P], fp32, name="ident")
            make_identity(nc, ident[:])
            identb = const.tile([P, P], bf16, name="identb")
            nc.vector.tensor_copy(identb[:], ident[:])
            onescol_f = const.tile([1, P], fp32, name="onescol_f")
            nc.vector.memset(onescol_f[:], 1.0)

            zeros_c = const.tile([P, 1], fp32, name="zeros_c")
            nc.vector.memset(zeros_c[:], 0.0)
            nc.const_aps.aps[(fp32, 0.0)] = zeros_c[:]
            eps_c = const.tile([P, 1], fp32, name="eps_c")
            nc.vector.memset(eps_c[:], BN_EPS)
            nc.const_aps.aps[(fp32, BN_EPS)] = eps_c[:]
            ones_c = const.tile([P, 1], fp32, name="ones_c")
            nc.vector.memset(ones_c[:], 1.0)
            nc.const_aps.aps[(fp32, 1.0)] = ones_c[:]

            # persistent state
            h_sb = state.tile([P, NTILES, H], bf16, name="h_sb")
            afas_sb = state.tile([P, NTILES, H2], fp8, name="afas_sb")
            hT8_sb = state.tile([P, NTILES, 2, P], fp8, name="hT8_sb")
            WiT_dyn = state.tile([P, 2, H2], fp8, name="WiT_dyn")
            WjT_dyn = state.tile([P, 2, H2], fp8, name="WjT_dyn")
            brow_dyn = state.tile([1, H2], bf16, name="brow_dyn")

            def transpose_tile(t):
                # hT8_sb[:, t, k, :] = h_sb[:, t, kP:(k+1)P].T  (fp8)
                for k in range(2):
                    tps = misc_pool.tile([P, P], bf16, tag="mps", name="tps")
                    nc.tensor.transpose(
                        tps[:], h_sb[:, t, k * P:(k + 1) * P], identb[:])
                    nc.scalar.activation(hT8_sb[:, t, k, :], tps[:], AF.Copy)

            for _rep in range(reps):
                bfbs_bounce = [dram.tile([NPAD, H2], fp8, name=f"bfbs_bounce{l}_{_rep}")
                               for l in range(L)]
                bfbs_full = [dram.tile([TROWS, H2], fp8, addr_space="Shared",
                                       name=f"bfbs_full{l}_{_rep}") for l in range(L)]
                stats_bounce = dram.tile([1, H2], fp32, name=f"stats_bounce_{_rep}")
                stats_full = dram.tile([1, H2], fp32, addr_space="Shared",
                                       name=f"stats_full_{_rep}")
                g_bounce = dram.tile([NGX, H], fp32, name=f"g_bounce{_rep}")
                g_full = dram.tile([NGX, H], fp32, addr_space="Shared",
                                   name=f"g_full{_rep}")

                # ---------- phase H0: h = x @ W_emb^T (+ b_emb) ----------
                for t in range(NTILES):
                    xTt = stream.tile([P, 2, P], bf16, tag="xTt", name="xTt")
                    nc.sync.dma_start(xTt[:], xT[:, :, t * P:(t + 1) * P])
                    hps = misc_ps([P, H], "hps")
                    nc.tensor.matmul(hps[:], xTt[:, 0, :], W_embT_sb[:, 0, :],
                                     start=True, stop=False)
                    nc.tensor.matmul(hps[:], xTt[:, 1, :], W_embT_sb[:, 1, :],
                                     start=False, stop=not cfg["has_emb_bias"])
                    if cfg["has_emb_bias"]:
                        nc.tensor.matmul(hps[:], onescol_sb[:1, :],
                                         b_embrow_sb[:1, :],
                                         start=False, stop=True)
                    nc.scalar.activation(h_sb[:, t, :], hps[:], AF.Copy)
                    transpose_tile(t)

                # ---------- layers ----------
                for l in range(L):
                    last = (l == L - 1)
                    Wi_cur = WiT_sb[l] if l == 0 else WiT_dyn
                    Wj_cur = WjT_sb[l] if l == 0 else WjT_dyn
                    use_bias = cfg["has_brow"][l] or l > 0
                    bias_row = (brow_sb[l] if l == 0 else brow_dyn)

                    # --- node projection tables (fp8 DoubleRow) ---
                    for t in range(NTILES):
                        aps = misc_ps([P, H2], "aps")
                        nc.tensor.matmul(aps[:], hT8_sb[:, t, :, :], Wi_cur[:],
                                         start=True, stop=True, perf_mode=DR)
                        nc.scalar.activation(afas_sb[:, t, :], aps[:], AF.Copy)
                        bps = misc_ps([P, H2], "bps")
                        nc.tensor.matmul(bps[:], hT8_sb[:, t, :, :], Wj_cur[:],
                                         start=True, stop=not use_bias,
                                         perf_mode=DR)
                        if use_bias:
                            nc.tensor.matmul(bps[:], onescol_sb[:1, :],
                                             bias_row[:1, :],
                                             start=False, stop=True)
                        bt = work.tile([P, H2], fp8, tag="bt", name="bt")
                        nc.vector.tensor_copy(bt[:], bps[:])
                        nc.sync.dma_start(bfbs_bounce[l][t * P:(t + 1) * P, :], bt[:])

                    if sim:
                        nc.sync.dma_start(bfbs_full[l][0:NPAD, :],
                                          bfbs_bounce[l][:])
                    else:
                        nc.gpsimd.collective_compute(
                            "AllGather", OP.bypass, replica_groups=groups,
                            ins=[bfbs_bounce[l].opt()], outs=[bfbs_full[l].opt()])

                    # --- BN stat accumulators (PSUM, one bank) ---
                    st_ps = st_pool.tile([1, H2], fp32, name="st_ps")

                    # --- edge chunks, grouped by dst tile; tiles processed in
                    # groups of GROUP so gate activations amortize ACT table
                    # loads ---
                    DEC_TILES = 5

                    def phase_a(t):
                        decouple = t < DEC_TILES
                        kt = KT[t]
                        q0 = chunk_base[t]
                        att = stream.tile([64, 2, kt * P], fp8, tag="att",
                                          name="att",
                                          padded_shape=[64, 2, KTMAX * P])
                        nc.sync.dma_start(att[:], attrT8[:, :, q0 * P:(q0 + kt) * P])
                        ogt = stream.tile([P, kt, P], fp8, tag="ogt", name="ogt",
                                          padded_shape=[P, KTMAX, P])
                        nc.sync.dma_start(ogt[:], og[:, q0:q0 + kt, :])
                        osct = stream.tile([P, kt, P], bf16, tag="osct", name="osct",
                                           padded_shape=[P, KTMAX, P])
                        nc.sync.dma_start(osct[:], osc[:, q0:q0 + kt, :])
                        srt = stream.tile([P, kt], i32, tag="srt", name="srt",
                                          padded_shape=[P, KTMAX])
                        nc.sync.dma_start(srt[:], srow[:, q0:q0 + kt])

                        # per-chunk gathers ([P,1] offsets; wider offset
                        # APs mis-lower on HW)
                        srcg_t = work.tile([P, kt, H2], fp8, tag="srcg_t",
                                           bufs=2, name="srcg_t",
                                           padded_shape=[P, KTMAX, H2])
                        for i in range(kt):
                            nc.gpsimd.indirect_dma_start(
                                out=srcg_t[:, i, :], out_offset=None,
                                in_=bfbs_full[l][:],
                                in_offset=bass.IndirectOffsetOnAxis(
                                    ap=srt[:, i:i + 1], axis=0))

                        pre_t = work.tile([P, kt, H2], bf16, tag="pre_t", bufs=GROUP,
                                          name="pre_t",
                                          padded_shape=[P, KTMAX, H2])
                        for i in range(kt):
                            pre = pre_pool.tile([P, H2], fp32, name="pre")
                            nc.tensor.matmul(pre[:],
                                             att[:, :, i * P:(i + 1) * P],
                                             WeT_sb[l][:],
                                             start=True, stop=False,
                                             perf_mode=DR)
                            nc.tensor.matmul(pre[:], ogt[:, i, :],
                                             afas_sb[:, t, :],
                                             start=False, stop=True)
                            if decouple:
                                nc.vector.tensor_copy(pre_t[:, i, :], pre[:])
                            else:
                                nc.vector.tensor_tensor(
                                    out=pre_t[:, i, :], in0=pre[:],
                                    in1=srcg_t[:, i, :], op=OP.add)
                        if decouple:
                            nc.vector.tensor_tensor(
                                out=pre_t[:], in0=pre_t[:],
                                in1=srcg_t[:, 0:kt, :], op=OP.add)
                        return dict(kt=kt, osct=osct, pre_t=pre_t)

                    def gates_sig(t, d, after=None):
                        kt = d["kt"]
                        sg_t = work.tile([P, kt, H], bf16, tag="sg_t", bufs=GROUP,
                                         name="sg_t", padded_shape=[P, KTMAX, H])
                        si = nc.scalar.activation(sg_t[:], d["pre_t"][:, :, 0:H],
                                                  AF.Sigmoid, scale=-1.0)
                        if after is not None:
                            tile.add_dep_helper(si.ins, after.ins, False,
                                                "group ACT table usage")
                        d["sg_t"] = sg_t
                        return si

                    def gates_expln(t, d, after=None):
                        kt = d["kt"]
                        eb_t = work.tile([P, kt, H], bf16, tag="ebmsg", bufs=5,
                                         name="eb_t", padded_shape=[P, KTMAX, H])
                        ei_ = nc.scalar.activation(eb_t[:], d["pre_t"][:, :, H:],
                                                   AF.Exp)
                        if after is not None:
                            tile.add_dep_helper(ei_.ins, after.ins, False,
                                                "group ACT table usage")
                        v_t = work.tile([P, kt, H], bf16, tag="v_t", bufs=2,
                                        name="v_t", padded_shape=[P, KTMAX, H])
                        li = nc.scalar.activation(v_t[:], eb_t[:], AF.Ln, bias=1.0)
                        msg_t = work.tile([P, kt, H], bf16, tag="ebmsg", bufs=5,
                                          name="msg_t", padded_shape=[P, KTMAX, H])
                        nc.vector.tensor_tensor(out=msg_t[:], in0=d["sg_t"],
                                                in1=v_t[:], op=OP.mult)
                        d["msg_t"] = msg_t
                        return li

                    def scatter_fin(t, d):
                        kt = d["kt"]
                        agg = agg_pool.tile([P, H], fp32, name="agg")
                        for i in range(kt):
                            nc.tensor.matmul(agg[:], d["osct"][:, i, :],
                                             d["msg_t"][:, i, :],
                                             start=(i == 0), stop=(i == kt - 1))
                        # h += agg * inv_deg ; then BN partial stats
                        nc.vector.scalar_tensor_tensor(
                            out=h_sb[:, t, :], in0=agg[:],
                            scalar=invdeg_sb[:, t:t + 1], in1=h_sb[:, t, :],
                            op0=OP.mult, op1=OP.add)
                        hh = work.tile([P, H2], bf16, tag="hh", bufs=3, name="hh")
                        nc.vector.tensor_copy(hh[:, :H], h_sb[:, t, :])
                        nc.scalar.activation(hh[:, H:], h_sb[:, t, :], AF.Square)
                        nc.tensor.matmul(st_ps[:], onesmask_sb[:, t:t + 1],
                                         hh[:],
                                         start=(t == 0), stop=(t == NTILES - 1))

                    prev_ln = None
                    for g0 in range(0, NTILES, GROUP):
                        ts = list(range(g0, min(g0 + GROUP, NTILES)))
                        ds = [phase_a(t) for t in ts]
                        sis = [gates_sig(t, d, after=prev_ln)
                               for t, d in zip(ts, ds)]
                        for t, d in zip(ts, ds):
                            prev_ln = gates_expln(t, d, after=sis[-1])
                            scatter_fin(t, d)

                    # --- BN stats -> AllReduce; compute affine A||B ---
                    if not last:
                        stats_acc = work.tile([1, H2], fp32, tag="small", bufs=1,
                                              name="stats_acc")
                        nc.vector.tensor_copy(stats_acc[:], st_ps[:])
                        nc.sync.dma_start(stats_bounce[:], stats_acc[:])
                        if sim:
                            nc.sync.dma_start(stats_full[:], stats_bounce[:])
                        else:
                            nc.gpsimd.collective_compute(
                                "AllReduce", OP.add, replica_groups=groups,
                                ins=[stats_bounce.opt()],
                                outs=[stats_full.opt()])
                        # transposes of raw h for next layer overlap the AR
                        for t in range(NTILES):
                            transpose_tile(t)
                        statsr = work.tile([1, H2], fp32, tag="small", bufs=1,
                                           name="statsr")
                        nc.sync.dma_start(statsr[:], stats_full[:])
                    else:
                        # readout pooling on raw h overlaps the AR; BN affine
                        # applied post-AR. stats ride rows [NG, NG+2).
                        gp = misc_ps([NG, H], "gp")
                        for t in range(NTILES):
                            nc.tensor.matmul(gp[:], oggraph_sb[:, t, :],
                                             h_sb[:, t, :],
                                             start=(t == 0),
                                             stop=(t == NTILES - 1))
                        gp_sb = work.tile([NGX, H], fp32, tag="gp_sb", bufs=1,
                                          name="gp_sb")
                        nc.vector.memset(gp_sb[:], 0.0)
                        nc.vector.tensor_scalar(out=gp_sb[:NG, :], in0=gp[:],
                                                scalar1=invgsz_sb[:, :1],
                                                scalar2=None, op0=OP.mult)
                        nc.vector.tensor_copy(gp_sb[64:65, :], st_ps[:, :H])
                        nc.vector.tensor_copy(gp_sb[96:97, :], st_ps[:, H:])
                        nc.sync.dma_start(g_bounce[:], gp_sb[:])
                        if sim:
                            nc.sync.dma_start(g_full[:], g_bounce[:])
                        else:
                            nc.gpsimd.collective_compute(
                                "AllReduce", OP.add, replica_groups=groups,
                                ins=[g_bounce.opt()], outs=[g_full.opt()])
                        statsr_g = work.tile([NGX, H], fp32, tag="gr", bufs=1,
                                             name="gr")
                        nc.sync.dma_start(statsr_g[:], g_full[:])

                    # mu = sum/N ; ex2 = sq/N ; var = ex2 - mu^2
                    ab = work.tile([1, H2], fp32, tag="small2", bufs=1, name="ab")
                    mu = work.tile([1, H], fp32, tag="small3", name="mu")
                    var = work.tile([1, H], fp32, tag="small3", name="var")
                    musq = work.tile([1, H], fp32, tag="small3", name="musq")
                    sd = work.tile([1, H], fp32, tag="small3", name="sd")
                    rsd = work.tile([1, H], fp32, tag="small3", name="rsd")
                    if not last:
                        nc.vector.tensor_scalar_mul(mu[:], statsr[:, :H], 1.0 / N)
                        nc.vector.tensor_scalar_mul(var[:], statsr[:, H:], 1.0 / N)
                    else:
                        nc.vector.tensor_scalar_mul(
                            mu[:], statsr_g[64:65, :], 1.0 / N)
                        nc.vector.tensor_scalar_mul(
                            var[:], statsr_g[96:97, :], 1.0 / N)
                    nc.vector.tensor_tensor(out=musq[:], in0=mu[:], in1=mu[:],
                                            op=OP.mult)
                    nc.vector.tensor_tensor(out=var[:], in0=var[:], in1=musq[:],
                                            op=OP.subtract)
                    nc.scalar.activation(sd[:], var[:], AF.Sqrt, bias=BN_EPS)
                    nc.vector.reciprocal(rsd[:], sd[:])
                    nc.vector.tensor_tensor(out=ab[:, :H], in0=rsd[:],
                                            in1=gamma_sb[l][:], op=OP.mult)
                    nc.vector.tensor_tensor(out=ab[:, H:], in0=mu[:],
                                            in1=ab[:, :H], op=OP.mult)
                    nc.vector.tensor_tensor(out=ab[:, H:], in0=beta_sb[l][:],
                                            in1=ab[:, H:], op=OP.subtract)

                    if not last:
                        # fold BN into the next layer's projections:
                        # A2[p, k] = A[k*128+p], B2 likewise (transposed views)
                        A2 = work.tile([P, 2, 1], fp32, tag="A2", bufs=1, name="A2")
                        B2 = work.tile([P, 2, 1], bf16, tag="B2", bufs=1, name="B2")
                        for k in range(2):
                            tp = misc_ps([P, 1], "tpA")
                            nc.tensor.transpose(tp[:], ab[:1, k * P:(k + 1) * P],
                                                ident[:1, :1])
                            nc.scalar.activation(A2[:, k, :], tp[:], AF.Copy)
                            tpb = misc_ps([P, 1], "tpB")
                            nc.tensor.transpose(tpb[:],
                                                ab[:1, H + k * P:H + (k + 1) * P],
                                                ident[:1, :1])
                            nc.scalar.activation(B2[:, k, :], tpb[:], AF.Copy)
                        for k in range(2):
                            nc.vector.tensor_scalar(
                                out=WiT_dyn[:, k, :], in0=WiT_sb[l + 1][:, k, :],
                                scalar1=A2[:, k, :1], scalar2=None, op0=OP.mult)
                            nc.vector.tensor_scalar(
                                out=WjT_dyn[:, k, :], in0=WjT_sb[l + 1][:, k, :],
                                scalar1=A2[:, k, :1], scalar2=None, op0=OP.mult)
                        brow_ps = misc_ps([1, H2], "brow_ps")
                        for k in range(2):
                            nc.tensor.matmul(brow_ps[:], B2[:, k, :],
                                             Wsum1_sb[:, k, :],
                                             start=(k == 0), stop=(k == 1))
                        if cfg["has_brow"][l + 1]:
                            nc.vector.tensor_tensor(out=brow_dyn[:],
                                                    in0=brow_ps[:],
                                                    in1=brow_sb[l + 1][:],
                                                    op=OP.add)
                        else:
                            nc.vector.tensor_copy(brow_dyn[:], brow_ps[:])
                        # lazy BN apply on h (off the critical path; must land
                        # before the next layer's scatter_fin / stats)
                        abps = misc_ps([P, H2], "abps")
                        nc.tensor.matmul(abps[:], onescol_f[:1, :], ab[:1, :],
                                         start=True, stop=True)
                        abb = work.tile([P, H2], fp32, tag="abb", bufs=1,
                                        name="abb")
                        nc.scalar.activation(abb[:], abps[:], AF.Copy)
                        for t in range(NTILES):
                            nc.vector.tensor_tensor(out=h_sb[:, t, :],
                                                    in0=h_sb[:, t, :],
                                                    in1=abb[:, :H], op=OP.mult)
                            nc.vector.tensor_tensor(out=h_sb[:, t, :],
                                                    in0=h_sb[:, t, :],
                                                    in1=abb[:, H:], op=OP.add)
                    else:
                        # readout: g_bn = A*g + B, then softplus/fc/softplus
                        abps = misc_ps([P, H2], "abps")
                        nc.tensor.matmul(abps[:], onescol_f[:1, :], ab[:1, :],
                                         start=True, stop=True)
                        abb = work.tile([P, H2], fp32, tag="abb", bufs=1,
                                        name="abb")
                        nc.scalar.activation(abb[:], abps[:], AF.Copy)
                        gbn = work.tile([NG, H], fp32, tag="gbn", bufs=1,
                                        name="gbn")
                        nc.vector.tensor_tensor(out=gbn[:], in0=statsr_g[:NG, :],
                                                in1=abb[:NG, :H], op=OP.mult)
                        nc.vector.tensor_tensor(out=gbn[:], in0=gbn[:],
                                                in1=abb[:NG, H:], op=OP.add)
                        ge = work.tile([NG, H], fp32, tag="ge", bufs=1, name="ge")
                        nc.scalar.activation(ge[:], gbn[:], AF.Exp)
                        spg = work.tile([NG, H], fp32, tag="spg", bufs=1,
                                        name="spg")
                        nc.scalar.activation(spg[:], ge[:], AF.Ln, bias=1.0)
                        spgT = work.tile([P, 2, NG], bf16, tag="spgT", bufs=1,
                                         name="spgT")
                        for k in range(2):
                            tp = misc_ps([P, NG], "tp")
                            nc.tensor.transpose(tp[:], spg[:, k * P:(k + 1) * P],
                                                ident[:NG, :NG])
                            nc.scalar.activation(spgT[:, k, :], tp[:], AF.Copy)
                        ops_ = misc_ps([NG, H], "ops_")
                        nc.tensor.matmul(ops_[:], spgT[:, 0, :], W_fcT_sb[:, 0, :],
                                         start=True, stop=False)
                        nc.tensor.matmul(ops_[:], spgT[:, 1, :], W_fcT_sb[:, 1, :],
                                         start=False,
                                         stop=not cfg["has_fc_bias"])
                        if cfg["has_fc_bias"]:
                            nc.tensor.matmul(ops_[:], onescol_sb[:1, :NG],
                                             b_fcrow_sb[:1, :],
                                             start=False, stop=True)
                        oe = work.tile([NG, H], fp32, tag="oe", bufs=1, name="oe")
                        nc.scalar.activation(oe[:], ops_[:], AF.Exp)
                        out_sb = work.tile([NG, H], fp32, tag="out_sb", bufs=1,
                                           name="out_sb")
                        nc.scalar.activation(out_sb[:], oe[:], AF.Ln, bias=1.0)
                        nc.sync.dma_start(out[:], out_sb[:])

    nc.compile()
    return nc


def kernel(**inputs):
    from concourse import bass_utils

    cfg, shared, per_core = _prep(**inputs)
    nc = _build(cfg)

    in_maps = []
    for c in range(NCORES):
        m = dict(shared)
        m.update(per_core[c])
        in_maps.append(m)

    res = bass_utils.run_bass_kernel_spmd(
        nc, in_maps, core_ids=list(range(NCORES)))
    return np.asarray(res.results[0]["out"], np.float32)


if __name__ == "__main__":
    import reference
    inputs = reference.setup_inputs()
    inputs = {k: np.asarray(v) for k, v in inputs.items()}
    got = kernel(**inputs)
    exp = np.asarray(reference.reference(**reference.setup_inputs()))
    err = np.abs(got - exp).max() / max(np.abs(exp).max(), 1e-9)
    print("max abs rel err:", err)
